# revision 2
# baseline (speedup 1.0000x reference)
"""AttentionBlock (B=8, C=512, N=2048, 8 heads) on 8 TRN2 NeuronCores — v7.

Sharding: data-parallel over batch — one batch per core.

v7 structure: ALL projections run upfront (interleaved with the input DMA,
with ScalarE helping on psum-evacuation copies while it is otherwise idle),
then a pure attention steady state with zero projection interference —
 the PE's in-order queue plus psum-pool round-robin made every attempt to
overlap projection with attention stall the ACT engine (measured
4.6-22us per occurrence).

Attention (per pair of heads, per 512-col j-block, 16 i-steps):
  - S^T: one [128,1024] psum tile; two concurrent K=64 matmuls on PE row
    groups (0,0)/(64,0) (same-cycle pairing, ~2x).
  - exp on ScalarE (the bottleneck engine: 256 x [128,1024] = 267us)
    psum -> bf16 pt tiles.
  - PV: four K=64 matmuls per step as two concurrent row-group pairs into
    four psum accumulators; adjacent pair-slots touch disjoint tiles
    (same-tile writers closer than 2 slots fault the psum banks). PV
    trails exp by PV_LAG steps (head-of-line: a PV semaphore wait at the
    PE queue head would block the next S pair).
  - normalize: DVE copy+add merge, denominator row partition-broadcast by
    DMA, reciprocal_approx_fast, DVE multiply, DMA out. (gpsimd
    partition_broadcast and DVE reciprocal are avoided — measured 1-3us.)
All matmuls in the kernel use the 64x128 row-tiled PE mode — no mode
switches (each costs ~300ns + lost pairing).
"""

import os
from contextlib import ExitStack

import numpy as np

import concourse.bacc as bacc
import concourse.bass_utils as bass_utils
import concourse.mybir as mybir
import concourse.tile as tile

F32 = mybir.dt.float32
BF16 = mybir.dt.bfloat16
AF = mybir.ActivationFunctionType

B = 8
HEAD = 8
D = 64
C = 512
N = 2048
PAIRS = HEAD // 2
CT = C // 128
NT = N // 512
ST = N // 128
VW = 65
PV_LAG = int(os.environ.get("PV_LAG", "3"))


def _col_perm():
    """Column order for the host-side reordered W.T ([512, 1536])."""
    cols = []
    for p in range(PAIRS):
        h0, h1 = 2 * p, 2 * p + 1
        cols += list(range(h0 * 192 + 64, h0 * 192 + 128))
        cols += list(range(h1 * 192 + 64, h1 * 192 + 128))
        cols += list(range(h0 * 192, h0 * 192 + 64))
        cols += list(range(h1 * 192, h1 * 192 + 64))
    for h in range(HEAD):
        cols += list(range(h * 192 + 128, h * 192 + 192))
    return np.array(cols, dtype=np.int64)


def build(repeat: int = 1, pt_bufs: int = 8):
    nc = bacc.Bacc("TRN2", target_bir_lowering=False, debug=False, num_devices=B)
    x_d = nc.dram_tensor("x", [C, N], F32, kind="ExternalInput").ap()
    wt_d = nc.dram_tensor("wt", [C, 3 * C], F32, kind="ExternalInput").ap()
    out_d = nc.dram_tensor("out", [C, N], F32, kind="ExternalOutput").ap()

    with tile.TileContext(nc) as tc:
        if repeat == 1:
            _emit(nc, tc, x_d, wt_d, out_d, pt_bufs)
        else:
            with tc.For_i(0, repeat, 1) as _i:
                _emit(nc, tc, x_d, wt_d, out_d, pt_bufs)
    nc.compile()
    return nc


def _emit(nc, tc, x_d, wt_d, out_d, pt_bufs):
    with ExitStack() as ctx:
        ctx.enter_context(nc.allow_low_precision(reason="bf16 attention"))
        persist = ctx.enter_context(tc.tile_pool(name="persist", bufs=1))
        stage = ctx.enter_context(tc.tile_pool(name="stage", bufs=4))
        qk_pool = ctx.enter_context(tc.tile_pool(name="qk", bufs=2 * PAIRS))
        pt_pool = ctx.enter_context(tc.tile_pool(name="pt", bufs=pt_bufs))
        sm_pool = ctx.enter_context(tc.tile_pool(name="small", bufs=8))
        ob_pool = ctx.enter_context(tc.tile_pool(name="ob", bufs=4))
        osb_pool = ctx.enter_context(tc.tile_pool(name="osb", bufs=8))
        mg_pool = ctx.enter_context(tc.tile_pool(name="mg", bufs=6))
        mix_ps = ctx.enter_context(tc.tile_pool(name="mix_ps", bufs=4,
                                                space="PSUM"))
        s_ps = ctx.enter_context(tc.tile_pool(name="s_ps", bufs=2, space="PSUM"))

        x_bf = persist.tile([128, CT, N], BF16, tag="xbf")
        wt_bf = persist.tile([128, CT, 3 * C], BF16, tag="wtbf")

        def ld_wt(ct, c0, c1):
            st = stage.tile([128, 512], F32, tag="st", name="st")
            nc.sync.dma_start(out=st[:, 0:c1 - c0],
                              in_=wt_d[ct * 128:(ct + 1) * 128, c0:c1])
            nc.vector.tensor_copy(out=wt_bf[:, ct, c0:c1], in_=st[:, 0:c1 - c0])

        def ld_x(ct, c0, c1):
            st = stage.tile([128, 512], F32, tag="st", name="st")
            nc.sync.dma_start(out=st[:, 0:c1 - c0],
                              in_=x_d[ct * 128:(ct + 1) * 128, c0:c1])
            nc.vector.tensor_copy(out=x_bf[:, ct, c0:c1], in_=st[:, 0:c1 - c0])

        vt_sb = persist.tile([128, ST, HEAD, VW], BF16, tag="vt")
        ones_sb = persist.tile([128, ST * HEAD], F32, tag="ones")
        nc.vector.memset(ones_sb, 1.0)
        nc.vector.tensor_copy(
            out=vt_sb[:, :, :, 64],
            in_=ones_sb.rearrange("p (s h) -> p s h", h=HEAD))

        def proj_pair(wcols, rhs_cols, lhs_is_x=False):
            """Accumulating K=64 row-group pair; returns (psA, psB)."""
            c0, c1 = wcols
            r0, r1 = rhs_cols
            w = r1 - r0
            psA = mix_ps.tile([128, w], F32, tag="mix", name="psA")
            psB = mix_ps.tile([128, w], F32, tag="mix", name="psB")
            for ct in range(CT):
                if lhs_is_x:
                    lA, lB = x_bf[0:64, ct, c0:c1], x_bf[64:128, ct, c0:c1]
                    rA, rB = wt_bf[0:64, ct, r0:r1], wt_bf[64:128, ct, r0:r1]
                else:
                    lA, lB = wt_bf[0:64, ct, c0:c1], wt_bf[64:128, ct, c0:c1]
                    rA, rB = x_bf[0:64, ct, r0:r1], x_bf[64:128, ct, r0:r1]
                nc.tensor.matmul(psA, lhsT=lA, rhs=rA,
                                 start=(ct == 0), stop=(ct == CT - 1))
                nc.tensor.matmul(psB, lhsT=lB, rhs=rB,
                                 start=(ct == 0), stop=(ct == CT - 1))
            return psA, psB

        def merge(out_ap, psA, psB, w, rearr=False, act_copy=False):
            # The psum evacuation copy runs on ScalarE only when it gates
            # the start of attention (ScalarE idle, DVE busy with casts);
            # otherwise DVE (copies ahead of exp in ScalarE's in-order
            # queue delay the whole pipeline). DVE does the add.
            tmp = mg_pool.tile([128, w], F32, tag="mg", name="tmp")
            (nc.scalar.copy if act_copy else nc.vector.tensor_copy)(tmp, psA)
            if rearr:
                nc.vector.tensor_add(
                    out_ap,
                    tmp.rearrange("p (h d) -> p h d", h=HEAD),
                    psB.rearrange("p (h d) -> p h d", h=HEAD))
            else:
                nc.vector.tensor_add(out_ap, tmp, psB)

        def qk_piece(p, t, which, nt, act_copy=False):
            blk = p * 256 + which * 128
            psA, psB = proj_pair((blk, blk + 128), (nt * 512, (nt + 1) * 512))
            merge(t[:, nt * 512:(nt + 1) * 512], psA, psB, 512,
                  act_copy=act_copy)

        def vproj_piece(i, act_copy=False):
            psA, psB = proj_pair((i * 128, (i + 1) * 128), (1024, 1536),
                                 lhs_is_x=True)
            merge(vt_sb[:, i, :, 0:64], psA, psB, 512, rearr=True,
                  act_copy=act_copy)

        def phase_b_block(p, kt, qt, j):
            h0, h1 = 2 * p, 2 * p + 1
            oacc = [mix_ps.tile([65, 512], F32, tag="mix", name=f"oacc{k}")
                    for k in range(4)]
            pts = [None] * ST
            for i in range(ST + PV_LAG):
                if i < ST:
                    sp = s_ps.tile([128, 1024], F32, tag="sps", name="sp")
                    nc.tensor.matmul(
                        sp[:, 0:512],
                        lhsT=kt[0:64, i * 128:(i + 1) * 128],
                        rhs=qt[0:64, j * 512:(j + 1) * 512],
                        start=True, stop=True)
                    nc.tensor.matmul(
                        sp[:, 512:1024],
                        lhsT=kt[64:128, i * 128:(i + 1) * 128],
                        rhs=qt[64:128, j * 512:(j + 1) * 512],
                        start=True, stop=True)
                    pt = pt_pool.tile([128, 1024], BF16, tag="pt", name="pt")
                    nc.scalar.activation(out=pt, in_=sp, func=AF.Exp,
                                         scale=1.0 / D)
                    pts[i] = pt
                if i >= PV_LAG:
                    ii = i - PV_LAG
                    pt = pts[ii]
                    first, last = (ii == 0), (ii == ST - 1)
                    nc.tensor.matmul(
                        oacc[0], lhsT=vt_sb[0:64, ii, h0, 0:65],
                        rhs=pt[0:64, 0:512], start=first, stop=last)
                    nc.tensor.matmul(
                        oacc[1], lhsT=vt_sb[64:128, ii, h0, 0:65],
                        rhs=pt[64:128, 0:512], start=first, stop=last)
                    nc.tensor.matmul(
                        oacc[2], lhsT=vt_sb[0:64, ii, h1, 0:65],
                        rhs=pt[0:64, 512:1024], start=first, stop=last)
                    nc.tensor.matmul(
                        oacc[3], lhsT=vt_sb[64:128, ii, h1, 0:65],
                        rhs=pt[64:128, 512:1024], start=first, stop=last)
            for h, oa, obp in ((h0, oacc[0], oacc[1]), (h1, oacc[2], oacc[3])):
                osb_a = osb_pool.tile([65, 512], F32, tag="osb", name="osba")
                nc.vector.tensor_copy(osb_a, oa)
                osb = osb_pool.tile([65, 512], F32, tag="osb", name="osb")
                nc.vector.tensor_add(osb, osb_a, obp)
                bden = sm_pool.tile([64, 512], F32, tag="bden", name="bden")
                nc.sync.dma_start(
                    out=bden,
                    in_=osb[64:65, :].rearrange("p (o t) -> p o t", o=1)
                    .to_broadcast([1, 64, 512]))
                rb = sm_pool.tile([64, 512], F32, tag="rb", name="rb")
                nc.vector.reciprocal_approx_fast(out=rb, in_=bden)
                ob = ob_pool.tile([64, 512], F32, tag="ob", name="ob")
                nc.vector.tensor_mul(ob, osb[0:64, :], rb)
                nc.sync.dma_start(
                    out=out_d[h * D:(h + 1) * D, j * 512:(j + 1) * 512],
                    in_=ob)

        # ---- prologue: DMA + ALL projections
        for ct in range(CT):
            ld_wt(ct, 0, 256)        # pair-0 qk
        for ct in range(CT):
            ld_wt(ct, 1024, 1536)    # v
        qk = [(qk_pool.tile([128, N], BF16, tag="qk", name=f"kt{p}"),
               qk_pool.tile([128, N], BF16, tag="qk", name=f"qt{p}"))
              for p in range(PAIRS)]
        for nt in range(NT):
            for ct in range(CT):
                ld_x(ct, nt * 512, (nt + 1) * 512)
            if nt == 0:
                for ct in range(CT):
                    ld_wt(ct, 256, 512)      # pair 1 qk
            if nt == 1:
                for ct in range(CT):
                    ld_wt(ct, 512, 1024)     # pairs 2-3 qk
            qk_piece(0, qk[0][0], 0, nt, act_copy=True)
            qk_piece(0, qk[0][1], 1, nt, act_copy=True)
            for i in range(nt * 4, (nt + 1) * 4):
                vproj_piece(i, act_copy=True)
        for p in range(1, PAIRS):
            for nt in range(NT):
                qk_piece(p, qk[p][0], 0, nt, act_copy=True)
                qk_piece(p, qk[p][1], 1, nt, act_copy=True)

        # ---- attention
        for p in range(PAIRS):
            for j in range(NT):
                phase_b_block(p, *qk[p], j)


_NC_CACHE = {}


def _get_nc(repeat=1):
    if repeat not in _NC_CACHE:
        _NC_CACHE[repeat] = build(repeat=repeat)
    return _NC_CACHE[repeat]


def kernel(x, W):
    """Full-input entry point: x [8,512,2048] f32, W [1536,512] f32 ->
    out [8,512,2048] f32. Shards batch over 8 cores internally."""
    x = np.asarray(x, dtype=np.float32)
    W = np.asarray(W, dtype=np.float32)
    assert x.shape == (B, C, N) and W.shape == (3 * C, C)
    nc = _get_nc()
    wt = np.ascontiguousarray(W.T[:, _col_perm()])
    in_maps = [{"x": np.ascontiguousarray(x[b]), "wt": wt} for b in range(B)]
    res = bass_utils.run_bass_kernel_spmd(nc, in_maps, core_ids=list(range(B)))
    return np.stack([res.results[b]["out"] for b in range(B)])
